# revision 10
# baseline (speedup 1.0000x reference)
"""TRN2 Bass kernel for nn_CommLayer (gnn message passing).

Math: x [B=65536, 512] viewed as [B, 8 agents, 64]; per agent a:
    y_a = tanh(x_a @ Wh.T + (sum_{a'!=a} x_{a'}) @ Wc.T / 7)
Rewritten with s = sum_a x_a:
    y_a = tanh(x_a @ WdT + s @ Wc7T),  WdT = Wh.T - Wc.T/7, Wc7T = Wc.T/7
a block-diagonal matmul plus a shared rank-64 term -- 7x less PE work
than the dense 512x512 matmul.

Everything runs in the TRANSPOSED domain in fp16 (max rel err ~6e-3 vs
tolerance 2e-2; bf16 fails at 4.7e-2). The host uploads per core
group-tiled x.T chunks and s.T, and reads back y.T group-tiled, so the
device never transposes: all DMA partition lines are 2-8 KiB
contiguous. fp16 halves HBM traffic (17 MiB/core total).

Engine plan per 1024-row group (8 groups/core):
  sync   : 2 input DMAs (x.T chunks 1.05 MB + s.T 0.13 MB)
  vector : duplicate s.T into partitions 64-127 (keeps every matmul
           k=128 -- half-array k=64 matmuls hold the PE's HAM clock
           gate at 1.2 GHz; full-width runs warm at 2.4 GHz)
  tensor : per chunk co: 2x(shared + blockdiag) fp16 matmuls F=512
           into a 2-bank PSUM tile; stationaries are constant weights
  scalar : 1 tanh [128, 1024] per chunk, PSUM -> fp16 SBUF
  gpsimd : 1 output DMA (1.05 MB)
PSUM: 4 tiles x 2 banks = all 8 banks, so the PE runs a full group
ahead of the activations.
"""
import sys

sys.path.insert(0, "/opt/trn_rl_repo")

import numpy as np

BATCH = 65536
D = 512
NAGENT = 8
DA = 64
NORM = NAGENT - 1
NCORES = 8
SHARD = BATCH // NCORES  # 8192
R = 1024                 # rows per group
NGROUP = SHARD // R      # 8
NCHUNK = D // 128        # 4
NMM = R // 512           # matmul F=512 slices per chunk

_CACHE: dict = {}


def _build_nc():
    import concourse.mybir as mybir
    import concourse.tile as tile
    from concourse import bacc

    nc = bacc.Bacc("TRN2", target_bir_lowering=False, debug=False)

    f16 = mybir.dt.float16
    f32 = mybir.dt.float32
    i8 = mybir.dt.int8

    x4_d = nc.dram_tensor(
        "x4", [NGROUP * 128, NCHUNK * R], f16, kind="ExternalInput"
    )
    st_d = nc.dram_tensor("st", [NGROUP * DA, R], f16, kind="ExternalInput")
    wd2_d = nc.dram_tensor("wd2", [128, 128], f16, kind="ExternalInput")
    wcs_d = nc.dram_tensor("wcs", [128, 128], f16, kind="ExternalInput")
    # tanh output is in [-1, 1]: ship it as int8 (x127) to halve the
    # store-side HBM traffic; quantization adds <4e-3 abs error
    y4_d = nc.dram_tensor(
        "y4", [NGROUP * 128, NCHUNK * R], i8, kind="ExternalOutput"
    )

    xv = x4_d[:].rearrange("(g p) f -> g p f", p=128)  # [8, 128, 4096]
    sv = st_d[:].rearrange("(g p) f -> g p f", p=DA)   # [8, 64, 1024]
    yv = y4_d[:].rearrange("(g p) f -> g p f", p=128)  # [8, 128, 4096]

    with tile.TileContext(nc) as tc:
        with (
            tc.tile_pool(name="const", bufs=1) as const,
            tc.tile_pool(name="xg", bufs=3) as xgp,
            tc.tile_pool(name="sg", bufs=3) as sgp,
            tc.tile_pool(name="og", bufs=3) as ogp,
            tc.tile_pool(name="oq", bufs=3) as oqp,
            tc.tile_pool(name="psy", bufs=4, space="PSUM") as psyp,
        ):
            xg_tiles = {}
            HALF = NCHUNK * R // 2

            def load_group(g, split=False):
                xg = xgp.tile([128, NCHUNK * R], f16, tag="xg", name=f"xg{g}")
                sg = sgp.tile([128, R], f16, tag="sg", name=f"sg{g}")
                if split:
                    # chunk-granular arrival so the first matmuls start
                    # as soon as chunk 0 lands
                    for c in range(NCHUNK):
                        nc.sync.dma_start(
                            xg[:, c * R:(c + 1) * R],
                            xv[g][:, c * R:(c + 1) * R],
                        )
                else:
                    nc.sync.dma_start(xg[:, 0:HALF], xv[g][:, 0:HALF])
                    nc.sync.dma_start(xg[:, HALF:], xv[g][:, HALF:])
                # s.T rides the store queue (balances the two HBM streams)
                nc.gpsimd.dma_start(sg[0:DA, :], sv[g])
                # duplicate s.T into the upper partition half (DVE is idle;
                # keeps the shared matmuls streaming all 128 partitions)
                nc.vector.tensor_copy(sg[DA:128, :], sg[0:DA, :])
                xg_tiles[g] = (xg, sg)

            # input loads first in program order: their DMA queues spin up
            # ~2.5us into the NEFF preamble, before the engine barrier ends
            load_group(0, split=True)
            load_group(1)
            wd2 = const.tile([128, 128], f16)
            nc.scalar.dma_start(wd2[:], wd2_d[:])
            wcs = const.tile([128, 128], f16)
            nc.scalar.dma_start(wcs[:], wcs_d[:])

            for g in range(NGROUP):
                if g + 2 < NGROUP:
                    load_group(g + 2)
                xg, sg = xg_tiles.pop(g)
                og = ogp.tile([128, NCHUNK * R], f16, tag="og", name=f"og{g}")
                oq = oqp.tile([128, NCHUNK * R], i8, tag="oq", name=f"oq{g}")
                for co in range(NCHUNK):
                    psy = psyp.tile([128, R], f32, tag="psy",
                                    name=f"psy{g}_{co}")
                    for h in range(NMM):
                        hs = slice(h * 512, (h + 1) * 512)
                        nc.tensor.matmul(
                            psy[:, hs], wcs[:], sg[:, hs],
                            start=True, stop=False,
                        )
                        nc.tensor.matmul(
                            psy[:, hs], wd2[:],
                            xg[:, co * R + h * 512:co * R + (h + 1) * 512],
                            start=False, stop=True,
                        )
                    nc.scalar.activation(
                        og[:, co * R:(co + 1) * R], psy[:],
                        mybir.ActivationFunctionType.Tanh,
                    )
                    # int8 quantization on the otherwise-idle DVE, then
                    # half-group stores alternating queues so the tail
                    # drains as results retire instead of one final burst
                    if co % 2 == 1:
                        hs2 = slice((co - 1) * R, (co + 1) * R)
                        nc.vector.tensor_scalar_mul(
                            oq[:, hs2], og[:, hs2], 127.0
                        )
                    if co == 1:
                        nc.gpsimd.dma_start(
                            yv[g][:, 0:HALF], oq[:, 0:HALF]
                        )
                    elif co == 3:
                        nc.scalar.dma_start(
                            yv[g][:, HALF:], oq[:, HALF:]
                        )

    nc.compile()
    return nc


def _get_nc():
    if "nc" not in _CACHE:
        _CACHE["nc"] = _build_nc()
    return _CACHE["nc"]


def _prepare_in_maps(inputs) -> list[dict]:
    """Full inputs -> per-core in_maps (host does transpose + fp16 cast)."""
    x = np.asarray(inputs["x"], dtype=np.float32)
    hw = np.asarray(inputs["hidden_weights"], dtype=np.float32)
    cw = np.asarray(inputs["communication_weights"], dtype=np.float32)
    assert x.shape == (BATCH, D), x.shape

    wc7t = cw.T / np.float32(NORM)          # [64, 64]
    wdt = hw.T - wc7t                       # [64, 64]
    wd2 = np.zeros((128, 128), dtype=np.float16)
    wd2[0:64, 0:64] = wdt
    wd2[64:128, 64:128] = wdt
    wcs = np.zeros((128, 128), dtype=np.float16)
    wcs[0:64, 0:64] = wc7t
    wcs[0:64, 64:128] = wc7t

    s = x.reshape(BATCH, NAGENT, DA).sum(axis=1)        # [B, 64] in f32
    x16 = x.astype(np.float16)
    s16 = s.astype(np.float16)

    in_maps = []
    for i in range(NCORES):
        rows = slice(i * SHARD, (i + 1) * SHARD)
        xt = x16[rows].T                                 # [512, 8192]
        st = s16[rows].T                                 # [64, 8192]
        # [4, 128, 8, 1024] -> [8, 128, 4, 1024]
        x4 = np.ascontiguousarray(
            xt.reshape(NCHUNK, 128, NGROUP, R).transpose(2, 1, 0, 3)
        ).reshape(NGROUP * 128, NCHUNK * R)
        stg = np.ascontiguousarray(
            st.reshape(DA, NGROUP, R).transpose(1, 0, 2)
        ).reshape(NGROUP * DA, R)
        in_maps.append({"x4": x4, "st": stg, "wd2": wd2, "wcs": wcs})
    return in_maps


def _decode_out(res) -> np.ndarray:
    y = np.empty((BATCH, D), dtype=np.float32)
    inv = np.float32(1.0 / 127.0)
    for i, r in enumerate(res.results):
        y4 = r["y4"].reshape(NGROUP, 128, NCHUNK, R)
        # y4[g, p, co, r] = 127 * y[g*R + r, co*128 + p]
        yi = y4.transpose(0, 3, 2, 1).reshape(SHARD, D)
        y[i * SHARD:(i + 1) * SHARD] = yi
    y *= inv
    return y


def kernel(**inputs) -> np.ndarray:
    from concourse.bass_utils import run_bass_kernel_spmd

    nc = _get_nc()
    in_maps = _prepare_in_maps(inputs)
    res = run_bass_kernel_spmd(nc, in_maps, core_ids=list(range(NCORES)))
    return _decode_out(res)
